# revision 14
# baseline (speedup 1.0000x reference)
"""Distributed Trainium2 kernel: pixel-shuffle -> W1 linear -> LayerNorm ->
vocab logits -> softmax -> expected token embedding.

Sharding: fully token-parallel (data-parallel over batch). Core c owns
batch c's 256 tokens end-to-end: phase A computes fhat for its tokens,
phase B computes logits against the FULL 32000-row vocab and contracts
P@E against the FULL embedding, streaming W2 (131MB) and the embedding
table (131MB) from HBM (~250GB/s/core, under the 358GB/s fair share).

This removes every collective from the previous vocab-parallel schedule:
no AllGather of activations (was ~60us of exposed PE idle), no
ReduceScatter of partial numerators (was ~41us of tail), no CC init
warmup, and no vocab zero-padding (250 exact v-tiles vs 2x32 padded,
~25us of padded matmuls).

Phase B loops over 10 chunks of 25 vocab tiles:
  logits:  per v-tile, 16 k-matmuls (F=256) -> PSUM [128v, 256t],
           exp on ScalarE -> pbar chunk in SBUF (bf16), DVE running
           row-sum r_acc for the softmax denominator.
  P@E:     per d-quarter: py[tt] accumulates 25 matmuls (F=512)
           lhsT=pbar tile, rhs=emb tile; DVE spill-add into an SBUF
           fp32 accumulator y_acc.
Softmax denominator finishes with two F=1 ones-matmuls (s = ones^T @
r_acc per token half); epilogue divides and DMAs out per (tt, dq) as
the last chunk's P@E completes.

Compute dtype: bf16 matmul inputs with fp32 PSUM accumulation; LayerNorm
and softmax statistics in fp32. No bf16 collective payloads anymore, so
the only error sources are the bf16 matmul operands themselves.
"""

import os
import sys
import types

import numpy as np
import ml_dtypes

N_CORES = 8
B, SEQ, DV = 8, 1024, 1152
DT = 2048          # text hidden size
V = 32000          # vocab
S = 2              # pixel shuffle scale
L = SEQ // (S * S)           # 256 tokens per batch after pixel shuffle
D4 = DV * S * S              # 4608
KA = D4 + 128                # contraction padded: +1 bias row, zero pad to 4736
KT = KA // 128               # 37 k-tiles for phase A
ET = DT // 128               # 16 e-tiles (contraction of logits)
VT = V // 128                # 250 vocab tiles
CH = 25                      # v-tiles per chunk
NCHUNK = VT // CH            # 10
LN_EPS = 1e-5

LAST_EXEC_TIME_NS = None

_BUILT = None


def _install_ntff_hook_shim():
    """bass_utils' trace path imports antenv.axon_hooks, which is absent in
    this image; provide it via sys.modules using the boot helper."""
    if "antenv.axon_hooks" in sys.modules:
        return
    try:
        from trn_agent_boot.trn_boot import _ntff_profile_via_ctypes

        hook = _ntff_profile_via_ctypes("/opt/axon/libaxon_pjrt.so")
        mod = types.ModuleType("antenv.axon_hooks")
        mod.get_axon_ntff_profile_hook = lambda: hook
        mod.set_axon_ntff_profile_hook = lambda h: None
        sys.modules["antenv.axon_hooks"] = mod
    except Exception:
        pass


def _build():
    import concourse.bass as bass  # noqa: F401
    import concourse.tile as tile
    from concourse import bacc, mybir
    from concourse.masks import make_identity

    f32 = mybir.dt.float32
    bf = mybir.dt.bfloat16
    AF = mybir.ActivationFunctionType
    ALU = mybir.AluOpType

    nc = bacc.Bacc("TRN2", target_bir_lowering=False, debug=False,
                   num_devices=N_CORES)

    xT = nc.declare_dram_parameter("xT", [KA, L], bf, isOutput=False)
    w1T = nc.declare_dram_parameter("w1T", [KA, DT], bf, isOutput=False)
    # w2s[p, vt, ko, v] = W2[vt*128+v, ko*128+p]; per-v-tile reads are
    # 4KB contiguous per partition
    w2s = nc.declare_dram_parameter("w2s", [128, VT * ET * 128], bf,
                                    isOutput=False)
    emb = nc.declare_dram_parameter("emb", [V, DT], bf, isOutput=False)
    out = nc.declare_dram_parameter("out", [L, DT], f32, isOutput=True)
    # scratch target for the emb-stream gate DMA (see below)
    embgate = nc.dram_tensor("embgate", [128, 1], bf)

    xT_ap = xT.ap().rearrange("(ko p) t -> p ko t", p=128)       # [128,37,256]
    w1T_ap = w1T.ap().rearrange("(ko p) e -> p ko e", p=128)     # [128,37,2048]
    w2s_ap = w2s.ap().rearrange("p (vt k) -> p vt k", k=ET * 128)
    emb_ap = emb.ap().rearrange("(vt p) d -> p vt d", p=128)     # [128,250,2048]

    with tile.TileContext(nc) as tc:
        # phase-B SBUF pools open BEFORE phase A's so they get disjoint
        # addresses: their DMAs are then gated only by queue position, not
        # by phase-A tiles' last readers -- the w2/emb streams start during
        # phase A instead of after its last matmul
        with tc.tile_pool(name="const", bufs=1) as constp, \
             tc.tile_pool(name="fTp", bufs=1) as fTp, \
             tc.tile_pool(name="w2p", bufs=9) as w2p, \
             tc.tile_pool(name="ep", bufs=6) as epool, \
             tc.tile_pool(name="pbp", bufs=2) as pbp, \
             tc.tile_pool(name="rp", bufs=1) as rp, \
             tc.tile_pool(name="yap", bufs=1) as yap, \
             tc.tile_pool(name="fin", bufs=2) as finp:
            identity = constp.tile([128, 128], bf)
            make_identity(nc, identity)
            eps_t = constp.tile([128, 1], f32)
            nc.vector.memset(eps_t, LN_EPS)
            ones_t = constp.tile([128, 1], bf)
            nc.vector.memset(ones_t, 1.0)

            # PE warmup: junk matmuls so HAM unthrottles before the
            # DMA-fed phase A matmuls arrive
            with tc.tile_pool(name="psW", bufs=1, space="PSUM") as psW:
                junk = psW.tile([128, 128], f32)
                for _ in range(40):
                    nc.tensor.matmul(junk, lhsT=identity, rhs=identity,
                                     start=True, stop=True,
                                     skip_group_check=True)

            # fhatT persists through phase B: [e-slice 128, ko, t]
            fhatT = fTp.tile([128, ET, L], bf)

            # ------------- phase A: f = pixshuf(x) @ W1^T + b, LayerNorm ----
            with tc.tile_pool(name="sbA", bufs=1) as sbA, \
                 tc.tile_pool(name="w1p", bufs=3) as w1p:
                xT_sb = sbA.tile([128, KT, L], bf)
                w1c0 = w1p.tile([128, 4, DT], bf, tag="w1c", name="w1c0")
                # first k-tile's operands ride ahead so MM(k=0) starts early
                # xT rides the scalar+gpsimd queues so the sync queue carries
                # only the w1 stream (phase A is DMA-paced at ~300GB/s)
                nc.scalar.dma_start(xT_sb[:, 0:2, :], xT_ap[:, 0:2, :])
                nc.sync.dma_start(w1c0[:, 0:1, :], w1T_ap[:, 0:1, :])
                nc.scalar.dma_start(xT_sb[:, 2:19, :], xT_ap[:, 2:19, :])
                nc.sync.dma_start(w1c0[:, 1:4, :], w1T_ap[:, 1:4, :])
                nc.gpsimd.dma_start(xT_sb[:, 19:KT, :], xT_ap[:, 19:KT, :])

                fhat = [[None, None], [None, None]]

                def ln_stats(tt, pf_t):
                    stats = fTp.tile([128, 4, 6], f32, tag="stats",
                                     name=f"stats{tt}")
                    for eb in range(4):
                        nc.vector.bn_stats(out=stats[:, eb, :],
                                           in_=pf_t[:, eb, :])
                    mv = fTp.tile([128, 2], f32, tag=f"mv{tt}", name=f"mv{tt}")
                    nc.vector.bn_aggr(out=mv, in_=stats)
                    rstd = fTp.tile([128, 1], f32, tag=f"rstd{tt}",
                                    name=f"rstd{tt}")
                    nc.scalar.activation(out=rstd, in_=mv[:, 1:2],
                                         func=AF.Sqrt, bias=eps_t, scale=1.0)
                    nc.vector.reciprocal(out=rstd, in_=rstd)
                    negmr = fTp.tile([128, 1], f32, tag=f"negmr{tt}",
                                     name=f"negmr{tt}")
                    nc.vector.tensor_scalar(
                        out=negmr, in0=mv[:, 0:1], scalar1=rstd,
                        scalar2=-1.0, op0=ALU.mult, op1=ALU.mult)
                    return mv, rstd, negmr

                def ln_norm(tt, eh, pf_t, mv, rstd, negmr):
                    ft = fTp.tile([128, 1024], bf, tag=f"fhat{tt}{eh}",
                                  name=f"fhat{tt}{eh}")
                    nc.vector.tensor_scalar(
                        out=ft[:, 0:512], in0=pf_t[:, eh * 2, :],
                        scalar1=mv[:, 0:1], scalar2=rstd,
                        op0=ALU.subtract, op1=ALU.mult)
                    nc.scalar.activation(
                        out=ft[:, 512:1024], in_=pf_t[:, eh * 2 + 1, :],
                        func=AF.Identity, scale=rstd, bias=negmr)
                    fhat[tt][eh] = ft

                def transp(tt, eh, psT):
                    for es in range(8):
                        pt = psT.tile([128, 128], bf, tag="pt",
                                      name=f"pt{tt}{eh}{es}")
                        nc.tensor.transpose(
                            pt, fhat[tt][eh][:, es * 128:(es + 1) * 128],
                            identity)
                        nc.vector.tensor_copy(
                            out=fhatT[:, eh * 8 + es,
                                      tt * 128:(tt + 1) * 128],
                            in_=pt)

                with tc.tile_pool(name="psA1", bufs=1, space="PSUM") as psA1:
                    pf1 = psA1.tile([128, 4, 512], f32, tag="pf1")
                    with tc.tile_pool(name="psA0", bufs=1,
                                      space="PSUM") as psA0:
                        pf0 = psA0.tile([128, 4, 512], f32, tag="pf0")
                        pf = [pf0, pf1]
                        w1cs = {}
                        for ck in range(10):  # chunks of 4 k-tiles (last: 1)
                            n = 4 if ck < 9 else 1
                            if ck == 0:
                                w1c = w1c0
                            else:
                                w1c = w1p.tile([128, 4, DT], bf, tag="w1c",
                                               name=f"w1c{ck}")
                                nc.sync.dma_start(
                                    w1c[:, :n, :],
                                    w1T_ap[:, ck * 4:ck * 4 + n, :])
                            w1cs[ck] = w1c
                            # last 5 k-tiles: tt0 first so LN(tt0) overlaps
                            # tt1's remaining matmuls (shrinks the PE-idle
                            # window before the transposes)
                            tts = (0,) if ck >= 8 else (0, 1)
                            for kk in range(n):
                                k = ck * 4 + kk
                                for tt in tts:
                                    for eb in range(4):
                                        nc.tensor.matmul(
                                            pf[tt][:, eb, :],
                                            lhsT=xT_sb[:, k,
                                                       tt * 128:(tt + 1) * 128],
                                            rhs=w1c[:, kk,
                                                    eb * 512:(eb + 1) * 512],
                                            start=(k == 0), stop=(k == KT - 1),
                                        )
                        # gate the emb stream: the et DMAs sit behind this
                        # read of the last w1 chunk on the gpsimd FIFO, so
                        # the emb prefetch starts only once the w1 stream is
                        # done instead of starving phase A at t~10
                        nc.gpsimd.dma_start(embgate.ap(),
                                            w1cs[9][:, 0, 0:1])
                        for ck in (8, 9):
                            n = 4 if ck < 9 else 1
                            for kk in range(n):
                                k = ck * 4 + kk
                                for eb in range(4):
                                    nc.tensor.matmul(
                                        pf[1][:, eb, :],
                                        lhsT=xT_sb[:, k, 128:256],
                                        rhs=w1cs[ck][:, kk,
                                                     eb * 512:(eb + 1) * 512],
                                        start=(k == 0), stop=(k == KT - 1),
                                    )
                        mv0, rstd0, negmr0 = ln_stats(0, pf0)
                        ln_norm(0, 0, pf0, mv0, rstd0, negmr0)
                        ln_norm(0, 1, pf0, mv0, rstd0, negmr0)
                    mv1, rstd1, negmr1 = ln_stats(1, pf1)
                    ln_norm(1, 0, pf1, mv1, rstd1, negmr1)
                    ln_norm(1, 1, pf1, mv1, rstd1, negmr1)
                    with tc.tile_pool(name="psT0", bufs=2,
                                      space="PSUM") as psT0:
                        transp(0, 0, psT0)
                        transp(0, 1, psT0)
                        transp(1, 0, psT0)
                        transp(1, 1, psT0)

            # ------------- phase B: logits -> exp -> s, y ------------------
            with tc.tile_pool(name="psL", bufs=2, space="PSUM") as psL, \
                 tc.tile_pool(name="psY", bufs=4, space="PSUM") as psY, \
                 tc.tile_pool(name="psS", bufs=1, space="PSUM") as psS:
                r_acc = rp.tile([128, L], f32)
                y_acc = yap.tile([128, 2, 4, 512], f32)  # [t-part, tt, dq, d]
                rec_sb = rp.tile([128, 2], f32, tag="rec")
                pbar_t = [None, None]

                def logits_chunk(ci, v0, sz):
                    pbar = pbar_t[ci % 2]
                    for i in range(sz):
                        vt = v0 + i
                        w2t = w2p.tile([128, ET, 128], bf, tag="w2t",
                                       name=f"w2t{vt}")
                        nc.sync.dma_start(
                            w2t, w2s_ap[:, vt, :].rearrange(
                                "p (ko v) -> p ko v", v=128))
                        pl = psL.tile([128, L], f32, tag="pl",
                                      name=f"pl{vt}")
                        for ko in range(ET):
                            nc.tensor.matmul(
                                pl, lhsT=w2t[:, ko, :], rhs=fhatT[:, ko, :],
                                start=(ko == 0), stop=(ko == ET - 1))
                        nc.scalar.activation(out=pbar[:, i, :], in_=pl,
                                             func=AF.Exp, scale=1.0)
                        if vt == 0:
                            nc.vector.tensor_copy(out=r_acc,
                                                  in_=pbar[:, 0, :])
                        else:
                            nc.vector.tensor_tensor(
                                r_acc, pbar[:, i, :], r_acc, ALU.add)

                def s_finalize():
                    # finish softmax denominator: s[t] = ones^T @ r_acc
                    r_bf = rp.tile([128, L], bf, tag="r_bf")
                    nc.vector.tensor_copy(out=r_bf, in_=r_acc)
                    s_ps = psS.tile([128, 2], f32, tag="s_ps")
                    for tt in range(2):
                        nc.tensor.matmul(
                            s_ps[:, tt:tt + 1],
                            lhsT=r_bf[:, tt * 128:(tt + 1) * 128],
                            rhs=ones_t, start=True, stop=True)
                    nc.vector.reciprocal(out=rec_sb, in_=s_ps)

                def pe_group(ci, v0, sz, dq, first, last):
                    pbar = pbar_t[ci % 2]
                    ets = []
                    for g in range((sz + 4) // 5):
                        w = min(5, sz - 5 * g)
                        et = epool.tile([128, 5, 512], bf, tag="et",
                                        name=f"et{ci}{dq}{g}")
                        nc.gpsimd.dma_start(
                            et[:, 0:w, :],
                            emb_ap[:, v0 + 5 * g:v0 + 5 * g + w,
                                   dq * 512:(dq + 1) * 512])
                        ets.append(et)
                    py = [psY.tile([128, 512], f32, tag="py",
                                   name=f"py{ci}{dq}{tt}")
                          for tt in range(2)]
                    for i in range(sz):
                        for tt in range(2):
                            nc.tensor.matmul(
                                py[tt],
                                lhsT=pbar[:, i, tt * 128:(tt + 1) * 128],
                                rhs=ets[i // 5][:, i % 5, :],
                                start=(i == 0), stop=(i == sz - 1))
                    for tt in range(2):
                        if first:
                            nc.vector.tensor_copy(
                                out=y_acc[:, tt, dq, :], in_=py[tt])
                        elif not last:
                            nc.vector.tensor_tensor(
                                y_acc[:, tt, dq, :], py[tt],
                                y_acc[:, tt, dq, :], ALU.add)
                        else:
                            # epilogue: out = (y_acc + py) / s
                            yf = finp.tile([128, 512], f32, tag="yf",
                                           name=f"yf{dq}{tt}")
                            nc.vector.tensor_tensor(
                                yf, py[tt], y_acc[:, tt, dq, :], ALU.add)
                            osb = finp.tile([128, 512], f32, tag="osb",
                                            name=f"osb{dq}{tt}")
                            nc.vector.tensor_scalar_mul(
                                out=osb, in0=yf,
                                scalar1=rec_sb[:, tt:tt + 1])
                            # sync queue is idle by now (w2 done) and HWDGE
                            # has less fixed latency than the gpsimd SWDGE
                            nc.sync.dma_start(
                                out.ap()[tt * 128:(tt + 1) * 128,
                                         dq * 512:(dq + 1) * 512],
                                osb)

                # smaller first chunk softens the w2+emb DMA ramp right
                # after the transposes; smaller last chunk shortens the
                # exposed epilogue chain
                chunks = [(0, 15)] + [(15 + 25 * i, 25) for i in range(9)] \
                    + [(240, 10)]
                for ci, (v0, sz) in enumerate(chunks):
                    last = ci == len(chunks) - 1
                    pbar_t[ci % 2] = pbp.tile([128, CH, L], bf, tag="pbar",
                                              name=f"pbar{ci}")
                    logits_chunk(ci, v0, sz)
                    if last:
                        s_finalize()
                    for dq in range(4):
                        pe_group(ci, v0, sz, dq, ci == 0, last)

    nc.finalize()
    return nc


def _pixel_shuffle_np(x, s=S):
    b, seq, d = x.shape
    h = w = int(seq ** 0.5)
    x = x.reshape(b, h, w, d)
    x = x.reshape(b, h, w // s, d * s)
    x = x.transpose(0, 2, 1, 3)
    x = x.reshape(b, w // s, h // s, d * s * s)
    x = x.transpose(0, 2, 1, 3)
    return x.reshape(b, seq // (s * s), d * s * s)


def kernel(vision_feats, llm_token_embed, W1_w, W1_b, W2_w):
    global _BUILT, LAST_EXEC_TIME_NS
    _install_ntff_hook_shim()
    from concourse import bass_utils

    bf16 = ml_dtypes.bfloat16

    if _BUILT is None:
        _BUILT = _build()
    nc = _BUILT

    x = _pixel_shuffle_np(np.asarray(vision_feats, np.float32))  # (8,256,4608)

    w1T_h = np.zeros((KA, DT), bf16)
    w1T_h[:D4] = np.asarray(W1_w, np.float32).T.astype(bf16)
    w1T_h[D4] = np.asarray(W1_b, np.float32).astype(bf16)

    # w2s[p, vt, ko, v] = W2[vt*128+v, ko*128+p]
    w2s_h = np.ascontiguousarray(
        np.asarray(W2_w, np.float32).astype(bf16)
        .reshape(VT, 128, ET, 128).transpose(3, 0, 2, 1)
    ).reshape(128, VT * ET * 128)

    emb_h = np.asarray(llm_token_embed, np.float32).astype(bf16)

    in_maps = []
    for c in range(N_CORES):
        xT_h = np.zeros((KA, L), bf16)
        xT_h[:D4] = x[c].T.astype(bf16)
        xT_h[D4] = 1.0
        in_maps.append({"xT": xT_h, "w1T": w1T_h, "w2s": w2s_h, "emb": emb_h})

    trace = bool(os.environ.get("KERNEL_TRACE"))
    kwargs = {}
    if trace:
        import tempfile

        kwargs["trace"] = True
        base = os.environ.get("KERNEL_TRACE_DIR")
        if base:
            os.makedirs(base, exist_ok=True)
            kwargs["tmpdir"] = tempfile.mkdtemp(dir=base)
        print("trace dir:", kwargs.get("tmpdir"), file=sys.stderr)

    res = bass_utils.run_bass_kernel_spmd(
        nc, in_maps, core_ids=list(range(N_CORES)), **kwargs)
    LAST_EXEC_TIME_NS = res.exec_time_ns

    out_full = np.stack(
        [np.asarray(res.results[c]["out"]) for c in range(N_CORES)], axis=0)
    return out_full.astype(np.float32)


# revision 15
# speedup vs baseline: 1.0009x; 1.0009x over previous
"""Distributed Trainium2 kernel: pixel-shuffle -> W1 linear -> LayerNorm ->
vocab logits -> softmax -> expected token embedding.

Sharding: fully token-parallel (data-parallel over batch). Core c owns
batch c's 256 tokens end-to-end: phase A computes fhat for its tokens,
phase B computes logits against the FULL 32000-row vocab and contracts
P@E against the FULL embedding, streaming W2 (131MB) and the embedding
table (131MB) from HBM (~250GB/s/core, under the 358GB/s fair share).

This removes every collective from the previous vocab-parallel schedule:
no AllGather of activations (was ~60us of exposed PE idle), no
ReduceScatter of partial numerators (was ~41us of tail), no CC init
warmup, and no vocab zero-padding (250 exact v-tiles vs 2x32 padded,
~25us of padded matmuls).

Phase B loops over 10 chunks of 25 vocab tiles:
  logits:  per v-tile, 16 k-matmuls (F=256) -> PSUM [128v, 256t],
           exp on ScalarE -> pbar chunk in SBUF (bf16), DVE running
           row-sum r_acc for the softmax denominator.
  P@E:     per d-quarter: py[tt] accumulates 25 matmuls (F=512)
           lhsT=pbar tile, rhs=emb tile; DVE spill-add into an SBUF
           fp32 accumulator y_acc.
Softmax denominator finishes with two F=1 ones-matmuls (s = ones^T @
r_acc per token half); epilogue divides and DMAs out per (tt, dq) as
the last chunk's P@E completes.

Compute dtype: bf16 matmul inputs with fp32 PSUM accumulation; LayerNorm
and softmax statistics in fp32. No bf16 collective payloads anymore, so
the only error sources are the bf16 matmul operands themselves.
"""

import os
import sys
import types

import numpy as np
import ml_dtypes

N_CORES = 8
B, SEQ, DV = 8, 1024, 1152
DT = 2048          # text hidden size
V = 32000          # vocab
S = 2              # pixel shuffle scale
L = SEQ // (S * S)           # 256 tokens per batch after pixel shuffle
D4 = DV * S * S              # 4608
KA = D4 + 128                # contraction padded: +1 bias row, zero pad to 4736
KT = KA // 128               # 37 k-tiles for phase A
ET = DT // 128               # 16 e-tiles (contraction of logits)
VT = V // 128                # 250 vocab tiles
CH = 25                      # v-tiles per chunk
NCHUNK = VT // CH            # 10
LN_EPS = 1e-5

LAST_EXEC_TIME_NS = None

_BUILT = None


def _install_ntff_hook_shim():
    """bass_utils' trace path imports antenv.axon_hooks, which is absent in
    this image; provide it via sys.modules using the boot helper."""
    if "antenv.axon_hooks" in sys.modules:
        return
    try:
        from trn_agent_boot.trn_boot import _ntff_profile_via_ctypes

        hook = _ntff_profile_via_ctypes("/opt/axon/libaxon_pjrt.so")
        mod = types.ModuleType("antenv.axon_hooks")
        mod.get_axon_ntff_profile_hook = lambda: hook
        mod.set_axon_ntff_profile_hook = lambda h: None
        sys.modules["antenv.axon_hooks"] = mod
    except Exception:
        pass


def _build():
    import concourse.bass as bass  # noqa: F401
    import concourse.tile as tile
    from concourse import bacc, mybir
    from concourse.masks import make_identity

    f32 = mybir.dt.float32
    bf = mybir.dt.bfloat16
    AF = mybir.ActivationFunctionType
    ALU = mybir.AluOpType

    nc = bacc.Bacc("TRN2", target_bir_lowering=False, debug=False,
                   num_devices=N_CORES)

    xT = nc.declare_dram_parameter("xT", [KA, L], bf, isOutput=False)
    w1T = nc.declare_dram_parameter("w1T", [KA, DT], bf, isOutput=False)
    # w2s[p, vt, ko, v] = W2[vt*128+v, ko*128+p]; per-v-tile reads are
    # 4KB contiguous per partition
    w2s = nc.declare_dram_parameter("w2s", [128, VT * ET * 128], bf,
                                    isOutput=False)
    emb = nc.declare_dram_parameter("emb", [V, DT], bf, isOutput=False)
    out = nc.declare_dram_parameter("out", [L, DT], f32, isOutput=True)
    # scratch target for the emb-stream gate DMA (see below)
    embgate = nc.dram_tensor("embgate", [128, 1], bf)

    xT_ap = xT.ap().rearrange("(ko p) t -> p ko t", p=128)       # [128,37,256]
    w1T_ap = w1T.ap().rearrange("(ko p) e -> p ko e", p=128)     # [128,37,2048]
    w2s_ap = w2s.ap().rearrange("p (vt k) -> p vt k", k=ET * 128)
    emb_ap = emb.ap().rearrange("(vt p) d -> p vt d", p=128)     # [128,250,2048]

    with tile.TileContext(nc) as tc:
        # phase-B SBUF pools open BEFORE phase A's so they get disjoint
        # addresses: their DMAs are then gated only by queue position, not
        # by phase-A tiles' last readers -- the w2/emb streams start during
        # phase A instead of after its last matmul
        with tc.tile_pool(name="const", bufs=1) as constp, \
             tc.tile_pool(name="fTp", bufs=1) as fTp, \
             tc.tile_pool(name="w2p", bufs=9) as w2p, \
             tc.tile_pool(name="ep", bufs=6) as epool, \
             tc.tile_pool(name="pbp", bufs=2) as pbp, \
             tc.tile_pool(name="rp", bufs=1) as rp, \
             tc.tile_pool(name="yap", bufs=1) as yap, \
             tc.tile_pool(name="fin", bufs=2) as finp:
            identity = constp.tile([128, 128], bf)
            make_identity(nc, identity)
            eps_t = constp.tile([128, 1], f32)
            nc.vector.memset(eps_t, LN_EPS)
            ones_t = constp.tile([128, 1], bf)
            nc.vector.memset(ones_t, 1.0)

            # PE warmup: junk matmuls so HAM unthrottles before the
            # DMA-fed phase A matmuls arrive
            with tc.tile_pool(name="psW", bufs=1, space="PSUM") as psW:
                junk = psW.tile([128, 128], f32)
                for _ in range(40):
                    nc.tensor.matmul(junk, lhsT=identity, rhs=identity,
                                     start=True, stop=True,
                                     skip_group_check=True)

            # fhatT persists through phase B: [e-slice 128, ko, t]
            fhatT = fTp.tile([128, ET, L], bf)

            # ------------- phase A: f = pixshuf(x) @ W1^T + b, LayerNorm ----
            with tc.tile_pool(name="sbA", bufs=1) as sbA, \
                 tc.tile_pool(name="w1p", bufs=3) as w1p:
                xT_sb = sbA.tile([128, KT, L], bf)
                w1c0 = w1p.tile([128, 4, DT], bf, tag="w1c", name="w1c0")
                # first k-tile's operands ride ahead so MM(k=0) starts early
                # xT rides the scalar+gpsimd queues so the sync queue carries
                # only the w1 stream (phase A is DMA-paced at ~300GB/s)
                nc.scalar.dma_start(xT_sb[:, 0:2, :], xT_ap[:, 0:2, :])
                nc.sync.dma_start(w1c0[:, 0:1, :], w1T_ap[:, 0:1, :])
                nc.scalar.dma_start(xT_sb[:, 2:19, :], xT_ap[:, 2:19, :])
                nc.sync.dma_start(w1c0[:, 1:4, :], w1T_ap[:, 1:4, :])
                nc.gpsimd.dma_start(xT_sb[:, 19:KT, :], xT_ap[:, 19:KT, :])

                fhat = [[None, None], [None, None]]

                def ln_stats(tt, pf_t):
                    stats = fTp.tile([128, 4, 6], f32, tag="stats",
                                     name=f"stats{tt}")
                    for eb in range(4):
                        nc.vector.bn_stats(out=stats[:, eb, :],
                                           in_=pf_t[:, eb, :])
                    mv = fTp.tile([128, 2], f32, tag=f"mv{tt}", name=f"mv{tt}")
                    nc.vector.bn_aggr(out=mv, in_=stats)
                    negmu = fTp.tile([128, 1], f32, tag=f"negmu{tt}",
                                     name=f"negmu{tt}")
                    nc.vector.tensor_scalar_mul(negmu, mv[:, 0:1], -1.0)
                    # rstd = exp(-0.5 ln(var+eps)) keeps the whole chain on
                    # the Scalar engine: one DVE->Scalar hop instead of
                    # bouncing Scalar-sqrt -> DVE-reciprocal -> Scalar
                    lnv = fTp.tile([128, 1], f32, tag=f"lnv{tt}",
                                   name=f"lnv{tt}")
                    nc.scalar.activation(out=lnv, in_=mv[:, 1:2],
                                         func=AF.Ln, bias=eps_t, scale=1.0)
                    rstd = fTp.tile([128, 1], f32, tag=f"rstd{tt}",
                                    name=f"rstd{tt}")
                    nc.scalar.activation(out=rstd, in_=lnv,
                                         func=AF.Exp, bias=0.0, scale=-0.5)
                    negmr = fTp.tile([128, 1], f32, tag=f"negmr{tt}",
                                     name=f"negmr{tt}")
                    nc.scalar.activation(out=negmr, in_=negmu,
                                         func=AF.Copy, bias=0.0, scale=rstd)
                    return mv, rstd, negmr

                def ln_norm(tt, eh, pf_t, mv, rstd, negmr):
                    ft = fTp.tile([128, 1024], bf, tag=f"fhat{tt}{eh}",
                                  name=f"fhat{tt}{eh}")
                    nc.vector.tensor_scalar(
                        out=ft[:, 0:512], in0=pf_t[:, eh * 2, :],
                        scalar1=mv[:, 0:1], scalar2=rstd,
                        op0=ALU.subtract, op1=ALU.mult)
                    nc.scalar.activation(
                        out=ft[:, 512:1024], in_=pf_t[:, eh * 2 + 1, :],
                        func=AF.Identity, scale=rstd, bias=negmr)
                    fhat[tt][eh] = ft

                def transp(tt, eh, psT):
                    for es in (4, 5, 6, 7, 0, 1, 2, 3):
                        pt = psT.tile([128, 128], bf, tag="pt",
                                      name=f"pt{tt}{eh}{es}")
                        nc.tensor.transpose(
                            pt, fhat[tt][eh][:, es * 128:(es + 1) * 128],
                            identity)
                        nc.vector.tensor_copy(
                            out=fhatT[:, eh * 8 + es,
                                      tt * 128:(tt + 1) * 128],
                            in_=pt)

                with tc.tile_pool(name="psA1", bufs=1, space="PSUM") as psA1:
                    pf1 = psA1.tile([128, 4, 512], f32, tag="pf1")
                    with tc.tile_pool(name="psA0", bufs=1,
                                      space="PSUM") as psA0:
                        pf0 = psA0.tile([128, 4, 512], f32, tag="pf0")
                        pf = [pf0, pf1]
                        w1cs = {}
                        for ck in range(10):  # chunks of 4 k-tiles (last: 1)
                            n = 4 if ck < 9 else 1
                            if ck == 0:
                                w1c = w1c0
                            else:
                                w1c = w1p.tile([128, 4, DT], bf, tag="w1c",
                                               name=f"w1c{ck}")
                                nc.sync.dma_start(
                                    w1c[:, :n, :],
                                    w1T_ap[:, ck * 4:ck * 4 + n, :])
                            w1cs[ck] = w1c
                            # last 5 k-tiles: tt0 first so LN(tt0) overlaps
                            # tt1's remaining matmuls (shrinks the PE-idle
                            # window before the transposes)
                            tts = (0,) if ck >= 7 else (0, 1)
                            for kk in range(n):
                                k = ck * 4 + kk
                                for tt in tts:
                                    for eb in range(4):
                                        nc.tensor.matmul(
                                            pf[tt][:, eb, :],
                                            lhsT=xT_sb[:, k,
                                                       tt * 128:(tt + 1) * 128],
                                            rhs=w1c[:, kk,
                                                    eb * 512:(eb + 1) * 512],
                                            start=(k == 0), stop=(k == KT - 1),
                                        )
                        # tt1 tail eb-major: each eb's pf1 slice
                        # finishes early enough for its bn_stats to overlap
                        # the remaining matmuls
                        for eb in range(4):
                            for ck in (7, 8, 9):
                                n = 4 if ck < 9 else 1
                                for kk in range(n):
                                    k = ck * 4 + kk
                                    nc.tensor.matmul(
                                        pf[1][:, eb, :],
                                        lhsT=xT_sb[:, k, 128:256],
                                        rhs=w1cs[ck][:, kk,
                                                     eb * 512:(eb + 1) * 512],
                                        start=(k == 0), stop=(k == KT - 1),
                                    )
                        mv0, rstd0, negmr0 = ln_stats(0, pf0)
                        # emb-stream gate: the et DMAs queue behind this
                        # gpsimd DMA, whose source is only written by the
                        # LN0 chain -- keeps the 3.8MB emb prefetch burst
                        # out of phase A's w1-stream window
                        nc.gpsimd.dma_start(embgate.ap()[0:1, :],
                                            negmr0[0:1, :])
                        ln_norm(0, 0, pf0, mv0, rstd0, negmr0)
                        ln_norm(0, 1, pf0, mv0, rstd0, negmr0)
                    mv1, rstd1, negmr1 = ln_stats(1, pf1)
                    ln_norm(1, 0, pf1, mv1, rstd1, negmr1)
                    ln_norm(1, 1, pf1, mv1, rstd1, negmr1)
                    with tc.tile_pool(name="psT0", bufs=2,
                                      space="PSUM") as psT0:
                        transp(0, 0, psT0)
                        transp(0, 1, psT0)
                        transp(1, 0, psT0)
                        transp(1, 1, psT0)

            # ------------- phase B: logits -> exp -> s, y ------------------
            with tc.tile_pool(name="psL", bufs=2, space="PSUM") as psL, \
                 tc.tile_pool(name="psY", bufs=4, space="PSUM") as psY, \
                 tc.tile_pool(name="psS", bufs=1, space="PSUM") as psS:
                r_acc = rp.tile([128, L], f32)
                y_acc = yap.tile([128, 2, 4, 512], f32)  # [t-part, tt, dq, d]
                rec_sb = rp.tile([128, 2], f32, tag="rec")
                pbar_t = [None, None]

                def logits_chunk(ci, v0, sz):
                    pbar = pbar_t[ci % 2]
                    for i in range(sz):
                        vt = v0 + i
                        w2t = w2p.tile([128, ET, 128], bf, tag="w2t",
                                       name=f"w2t{vt}")
                        nc.sync.dma_start(
                            w2t, w2s_ap[:, vt, :].rearrange(
                                "p (ko v) -> p ko v", v=128))
                        pl = psL.tile([128, L], f32, tag="pl",
                                      name=f"pl{vt}")
                        for ko in range(ET):
                            nc.tensor.matmul(
                                pl, lhsT=w2t[:, ko, :], rhs=fhatT[:, ko, :],
                                start=(ko == 0), stop=(ko == ET - 1))
                        nc.scalar.activation(out=pbar[:, i, :], in_=pl,
                                             func=AF.Exp, scale=1.0)
                        if vt == 0:
                            nc.vector.tensor_copy(out=r_acc,
                                                  in_=pbar[:, 0, :])
                        else:
                            nc.vector.tensor_tensor(
                                r_acc, pbar[:, i, :], r_acc, ALU.add)

                def s_finalize():
                    # finish softmax denominator: s[t] = ones^T @ r_acc
                    r_bf = rp.tile([128, L], bf, tag="r_bf")
                    nc.vector.tensor_copy(out=r_bf, in_=r_acc)
                    s_ps = psS.tile([128, 2], f32, tag="s_ps")
                    for tt in range(2):
                        nc.tensor.matmul(
                            s_ps[:, tt:tt + 1],
                            lhsT=r_bf[:, tt * 128:(tt + 1) * 128],
                            rhs=ones_t, start=True, stop=True)
                    nc.vector.reciprocal(out=rec_sb, in_=s_ps)

                def pe_group(ci, v0, sz, dq, first, last):
                    pbar = pbar_t[ci % 2]
                    ets = []
                    for g in range((sz + 4) // 5):
                        w = min(5, sz - 5 * g)
                        et = epool.tile([128, 5, 512], bf, tag="et",
                                        name=f"et{ci}{dq}{g}")
                        nc.gpsimd.dma_start(
                            et[:, 0:w, :],
                            emb_ap[:, v0 + 5 * g:v0 + 5 * g + w,
                                   dq * 512:(dq + 1) * 512])
                        ets.append(et)
                    py = [psY.tile([128, 512], f32, tag="py",
                                   name=f"py{ci}{dq}{tt}")
                          for tt in range(2)]
                    for i in range(sz):
                        for tt in range(2):
                            nc.tensor.matmul(
                                py[tt],
                                lhsT=pbar[:, i, tt * 128:(tt + 1) * 128],
                                rhs=ets[i // 5][:, i % 5, :],
                                start=(i == 0), stop=(i == sz - 1))
                    for tt in range(2):
                        if first:
                            nc.vector.tensor_copy(
                                out=y_acc[:, tt, dq, :], in_=py[tt])
                        elif not last:
                            nc.vector.tensor_tensor(
                                y_acc[:, tt, dq, :], py[tt],
                                y_acc[:, tt, dq, :], ALU.add)
                        else:
                            # epilogue: out = (y_acc + py) / s
                            yf = finp.tile([128, 512], f32, tag="yf",
                                           name=f"yf{dq}{tt}")
                            nc.vector.tensor_tensor(
                                yf, py[tt], y_acc[:, tt, dq, :], ALU.add)
                            osb = finp.tile([128, 512], f32, tag="osb",
                                            name=f"osb{dq}{tt}")
                            nc.vector.tensor_scalar_mul(
                                out=osb, in0=yf,
                                scalar1=rec_sb[:, tt:tt + 1])
                            # sync queue is idle by now (w2 done) and HWDGE
                            # has less fixed latency than the gpsimd SWDGE
                            nc.sync.dma_start(
                                out.ap()[tt * 128:(tt + 1) * 128,
                                         dq * 512:(dq + 1) * 512],
                                osb)

                # smaller first chunk softens the w2+emb DMA ramp right
                # after the transposes; smaller last chunk shortens the
                # exposed epilogue chain
                chunks = [(0, 15)] + [(15 + 25 * i, 25) for i in range(9)] \
                    + [(240, 10)]
                for ci, (v0, sz) in enumerate(chunks):
                    last = ci == len(chunks) - 1
                    pbar_t[ci % 2] = pbp.tile([128, CH, L], bf, tag="pbar",
                                              name=f"pbar{ci}")
                    logits_chunk(ci, v0, sz)
                    if last:
                        s_finalize()
                    for dq in range(4):
                        pe_group(ci, v0, sz, dq, ci == 0, last)

    nc.finalize()
    return nc


def _pixel_shuffle_np(x, s=S):
    b, seq, d = x.shape
    h = w = int(seq ** 0.5)
    x = x.reshape(b, h, w, d)
    x = x.reshape(b, h, w // s, d * s)
    x = x.transpose(0, 2, 1, 3)
    x = x.reshape(b, w // s, h // s, d * s * s)
    x = x.transpose(0, 2, 1, 3)
    return x.reshape(b, seq // (s * s), d * s * s)


def kernel(vision_feats, llm_token_embed, W1_w, W1_b, W2_w):
    global _BUILT, LAST_EXEC_TIME_NS
    _install_ntff_hook_shim()
    from concourse import bass_utils

    bf16 = ml_dtypes.bfloat16

    if _BUILT is None:
        _BUILT = _build()
    nc = _BUILT

    x = _pixel_shuffle_np(np.asarray(vision_feats, np.float32))  # (8,256,4608)

    w1T_h = np.zeros((KA, DT), bf16)
    w1T_h[:D4] = np.asarray(W1_w, np.float32).T.astype(bf16)
    w1T_h[D4] = np.asarray(W1_b, np.float32).astype(bf16)

    # w2s[p, vt, ko, v] = W2[vt*128+v, ko*128+p]
    w2s_h = np.ascontiguousarray(
        np.asarray(W2_w, np.float32).astype(bf16)
        .reshape(VT, 128, ET, 128).transpose(3, 0, 2, 1)
    ).reshape(128, VT * ET * 128)

    emb_h = np.asarray(llm_token_embed, np.float32).astype(bf16)

    in_maps = []
    for c in range(N_CORES):
        xT_h = np.zeros((KA, L), bf16)
        xT_h[:D4] = x[c].T.astype(bf16)
        xT_h[D4] = 1.0
        in_maps.append({"xT": xT_h, "w1T": w1T_h, "w2s": w2s_h, "emb": emb_h})

    trace = bool(os.environ.get("KERNEL_TRACE"))
    kwargs = {}
    if trace:
        import tempfile

        kwargs["trace"] = True
        base = os.environ.get("KERNEL_TRACE_DIR")
        if base:
            os.makedirs(base, exist_ok=True)
            kwargs["tmpdir"] = tempfile.mkdtemp(dir=base)
        print("trace dir:", kwargs.get("tmpdir"), file=sys.stderr)

    res = bass_utils.run_bass_kernel_spmd(
        nc, in_maps, core_ids=list(range(N_CORES)), **kwargs)
    LAST_EXEC_TIME_NS = res.exec_time_ns

    out_full = np.stack(
        [np.asarray(res.results[c]["out"]) for c in range(N_CORES)], axis=0)
    return out_full.astype(np.float32)


# revision 16
# speedup vs baseline: 1.0224x; 1.0215x over previous
"""Distributed Trainium2 kernel: pixel-shuffle -> W1 linear -> LayerNorm ->
vocab logits -> softmax -> expected token embedding.

Sharding: fully token-parallel (data-parallel over batch). Core c owns
batch c's 256 tokens end-to-end: phase A computes fhat for its tokens,
phase B computes logits against the FULL 32000-row vocab and contracts
P@E against the FULL embedding, streaming W2 (131MB) and the embedding
table (131MB) from HBM (~250GB/s/core, under the 358GB/s fair share).

This removes every collective from the previous vocab-parallel schedule:
no AllGather of activations (was ~60us of exposed PE idle), no
ReduceScatter of partial numerators (was ~41us of tail), no CC init
warmup, and no vocab zero-padding (250 exact v-tiles vs 2x32 padded,
~25us of padded matmuls).

Phase B loops over 10 chunks of 25 vocab tiles:
  logits:  per v-tile, 16 k-matmuls (F=256) -> PSUM [128v, 256t],
           exp on ScalarE -> pbar chunk in SBUF (bf16), DVE running
           row-sum r_acc for the softmax denominator.
  P@E:     per d-quarter: py[tt] accumulates 25 matmuls (F=512)
           lhsT=pbar tile, rhs=emb tile; DVE spill-add into an SBUF
           fp32 accumulator y_acc.
Softmax denominator finishes with two F=1 ones-matmuls (s = ones^T @
r_acc per token half); epilogue divides and DMAs out per (tt, dq) as
the last chunk's P@E completes.

Compute dtype: bf16 matmul inputs with fp32 PSUM accumulation; LayerNorm
and softmax statistics in fp32. No bf16 collective payloads anymore, so
the only error sources are the bf16 matmul operands themselves.
"""

import os
import sys
import types

import numpy as np
import ml_dtypes

N_CORES = 8
B, SEQ, DV = 8, 1024, 1152
DT = 2048          # text hidden size
V = 32000          # vocab
S = 2              # pixel shuffle scale
L = SEQ // (S * S)           # 256 tokens per batch after pixel shuffle
D4 = DV * S * S              # 4608
KA = D4 + 128                # contraction padded: +1 bias row, zero pad to 4736
KT = KA // 128               # 37 k-tiles for phase A
ET = DT // 128               # 16 e-tiles (contraction of logits)
VT = V // 128                # 250 vocab tiles
CH = 25                      # v-tiles per chunk
NCHUNK = VT // CH            # 10
LN_EPS = 1e-5

LAST_EXEC_TIME_NS = None

_BUILT = None


def _install_ntff_hook_shim():
    """bass_utils' trace path imports antenv.axon_hooks, which is absent in
    this image; provide it via sys.modules using the boot helper."""
    if "antenv.axon_hooks" in sys.modules:
        return
    try:
        from trn_agent_boot.trn_boot import _ntff_profile_via_ctypes

        hook = _ntff_profile_via_ctypes("/opt/axon/libaxon_pjrt.so")
        mod = types.ModuleType("antenv.axon_hooks")
        mod.get_axon_ntff_profile_hook = lambda: hook
        mod.set_axon_ntff_profile_hook = lambda h: None
        sys.modules["antenv.axon_hooks"] = mod
    except Exception:
        pass


def _build():
    import concourse.bass as bass  # noqa: F401
    import concourse.tile as tile
    from concourse import bacc, mybir
    from concourse.masks import make_identity

    f32 = mybir.dt.float32
    bf = mybir.dt.bfloat16
    AF = mybir.ActivationFunctionType
    ALU = mybir.AluOpType

    nc = bacc.Bacc("TRN2", target_bir_lowering=False, debug=False,
                   num_devices=N_CORES)

    # xTs[p, ko, t] = pixshuf(x).T[ko*128+p, t]; w1s[p, ko, e] likewise:
    # per-partition-contiguous so each DMA is ~128 fat descriptors instead
    # of thousands of 512B ones (descriptor processing was throttling the
    # phase-A stream to ~190GB/s)
    xT = nc.declare_dram_parameter("xT", [128, KT * L], bf, isOutput=False)
    w1T = nc.declare_dram_parameter("w1T", [128, KT * DT], bf,
                                    isOutput=False)
    # w2s[p, vt, ko, v] = W2[vt*128+v, ko*128+p]; per-v-tile reads are
    # 4KB contiguous per partition
    w2s = nc.declare_dram_parameter("w2s", [128, VT * ET * 128], bf,
                                    isOutput=False)
    # embs[p, dq, vt, d] = emb[vt*128+p, dq*512+d]: a 5-v-tile dq-slice
    # read is one contiguous 5KB run per partition
    emb = nc.declare_dram_parameter("emb", [128, 4 * VT * 512], bf,
                                    isOutput=False)
    out = nc.declare_dram_parameter("out", [L, DT], f32, isOutput=True)
    # scratch target for the emb-stream gate DMA (see below)
    embgate = nc.dram_tensor("embgate", [128, 1], bf)

    xT_ap = xT.ap().rearrange("p (ko t) -> p ko t", t=L)         # [128,37,256]
    w1T_ap = w1T.ap().rearrange("p (ko e) -> p ko e", e=DT)      # [128,37,2048]
    w2s_ap = w2s.ap().rearrange("p (vt k) -> p vt k", k=ET * 128)
    emb_ap = emb.ap().rearrange("p (dq vt d) -> p dq vt d", d=512, vt=VT)

    with tile.TileContext(nc) as tc:
        # phase-B SBUF pools open BEFORE phase A's so they get disjoint
        # addresses: their DMAs are then gated only by queue position, not
        # by phase-A tiles' last readers -- the w2/emb streams start during
        # phase A instead of after its last matmul
        with tc.tile_pool(name="const", bufs=1) as constp, \
             tc.tile_pool(name="fTp", bufs=1) as fTp, \
             tc.tile_pool(name="w2p", bufs=9) as w2p, \
             tc.tile_pool(name="ep", bufs=6) as epool, \
             tc.tile_pool(name="pbp", bufs=2) as pbp, \
             tc.tile_pool(name="rp", bufs=1) as rp, \
             tc.tile_pool(name="yap", bufs=1) as yap, \
             tc.tile_pool(name="fin", bufs=2) as finp:
            identity = constp.tile([128, 128], bf)
            make_identity(nc, identity)
            eps_t = constp.tile([128, 1], f32)
            nc.vector.memset(eps_t, LN_EPS)
            ones_t = constp.tile([128, 1], bf)
            nc.vector.memset(ones_t, 1.0)

            # PE warmup: junk matmuls so HAM unthrottles before the
            # DMA-fed phase A matmuls arrive
            with tc.tile_pool(name="psW", bufs=1, space="PSUM") as psW:
                junk = psW.tile([128, 128], f32)
                for _ in range(40):
                    nc.tensor.matmul(junk, lhsT=identity, rhs=identity,
                                     start=True, stop=True,
                                     skip_group_check=True)

            # fhatT persists through phase B: [e-slice 128, ko, t]
            fhatT = fTp.tile([128, ET, L], bf)

            # ------------- phase A: f = pixshuf(x) @ W1^T + b, LayerNorm ----
            with tc.tile_pool(name="sbA", bufs=1) as sbA, \
                 tc.tile_pool(name="w1p", bufs=3) as w1p:
                xT_sb = sbA.tile([128, KT, L], bf)
                w1c0 = w1p.tile([128, 4, DT], bf, tag="w1c", name="w1c0")
                # first k-tile's operands ride ahead so MM(k=0) starts early
                # xT rides the scalar+gpsimd queues so the sync queue carries
                # only the w1 stream (phase A is DMA-paced at ~300GB/s)
                nc.scalar.dma_start(xT_sb[:, 0:2, :], xT_ap[:, 0:2, :])
                nc.sync.dma_start(w1c0[:, 0:1, :], w1T_ap[:, 0:1, :])
                nc.scalar.dma_start(xT_sb[:, 2:19, :], xT_ap[:, 2:19, :])
                nc.sync.dma_start(w1c0[:, 1:4, :], w1T_ap[:, 1:4, :])
                nc.gpsimd.dma_start(xT_sb[:, 19:KT, :], xT_ap[:, 19:KT, :])

                fhat = [[None, None], [None, None]]

                def ln_stats(tt, pf_t):
                    stats = fTp.tile([128, 4, 6], f32, tag="stats",
                                     name=f"stats{tt}")
                    for eb in range(4):
                        nc.vector.bn_stats(out=stats[:, eb, :],
                                           in_=pf_t[:, eb, :])
                    mv = fTp.tile([128, 2], f32, tag=f"mv{tt}", name=f"mv{tt}")
                    nc.vector.bn_aggr(out=mv, in_=stats)
                    negmu = fTp.tile([128, 1], f32, tag=f"negmu{tt}",
                                     name=f"negmu{tt}")
                    nc.vector.tensor_scalar_mul(negmu, mv[:, 0:1], -1.0)
                    # rstd = exp(-0.5 ln(var+eps)) keeps the whole chain on
                    # the Scalar engine: one DVE->Scalar hop instead of
                    # bouncing Scalar-sqrt -> DVE-reciprocal -> Scalar
                    lnv = fTp.tile([128, 1], f32, tag=f"lnv{tt}",
                                   name=f"lnv{tt}")
                    nc.scalar.activation(out=lnv, in_=mv[:, 1:2],
                                         func=AF.Ln, bias=eps_t, scale=1.0)
                    rstd = fTp.tile([128, 1], f32, tag=f"rstd{tt}",
                                    name=f"rstd{tt}")
                    nc.scalar.activation(out=rstd, in_=lnv,
                                         func=AF.Exp, bias=0.0, scale=-0.5)
                    negmr = fTp.tile([128, 1], f32, tag=f"negmr{tt}",
                                     name=f"negmr{tt}")
                    nc.scalar.activation(out=negmr, in_=negmu,
                                         func=AF.Copy, bias=0.0, scale=rstd)
                    return mv, rstd, negmr

                def ln_norm(tt, eh, pf_t, mv, rstd, negmr):
                    ft = fTp.tile([128, 1024], bf, tag=f"fhat{tt}{eh}",
                                  name=f"fhat{tt}{eh}")
                    nc.vector.tensor_scalar(
                        out=ft[:, 0:512], in0=pf_t[:, eh * 2, :],
                        scalar1=mv[:, 0:1], scalar2=rstd,
                        op0=ALU.subtract, op1=ALU.mult)
                    nc.scalar.activation(
                        out=ft[:, 512:1024], in_=pf_t[:, eh * 2 + 1, :],
                        func=AF.Identity, scale=rstd, bias=negmr)
                    fhat[tt][eh] = ft

                def transp(tt, eh, psT):
                    for es in (4, 5, 6, 7, 0, 1, 2, 3):
                        pt = psT.tile([128, 128], bf, tag="pt",
                                      name=f"pt{tt}{eh}{es}")
                        nc.tensor.transpose(
                            pt, fhat[tt][eh][:, es * 128:(es + 1) * 128],
                            identity)
                        nc.vector.tensor_copy(
                            out=fhatT[:, eh * 8 + es,
                                      tt * 128:(tt + 1) * 128],
                            in_=pt)

                with tc.tile_pool(name="psA1", bufs=1, space="PSUM") as psA1:
                    pf1 = psA1.tile([128, 4, 512], f32, tag="pf1")
                    with tc.tile_pool(name="psA0", bufs=1,
                                      space="PSUM") as psA0:
                        pf0 = psA0.tile([128, 4, 512], f32, tag="pf0")
                        pf = [pf0, pf1]
                        w1cs = {}
                        for ck in range(10):  # chunks of 4 k-tiles (last: 1)
                            n = 4 if ck < 9 else 1
                            if ck == 0:
                                w1c = w1c0
                            else:
                                w1c = w1p.tile([128, 4, DT], bf, tag="w1c",
                                               name=f"w1c{ck}")
                                nc.sync.dma_start(
                                    w1c[:, :n, :],
                                    w1T_ap[:, ck * 4:ck * 4 + n, :])
                            w1cs[ck] = w1c
                            # last 5 k-tiles: tt0 first so LN(tt0) overlaps
                            # tt1's remaining matmuls (shrinks the PE-idle
                            # window before the transposes)
                            tts = (0,) if ck >= 7 else (0, 1)
                            for kk in range(n):
                                k = ck * 4 + kk
                                for tt in tts:
                                    for eb in range(4):
                                        nc.tensor.matmul(
                                            pf[tt][:, eb, :],
                                            lhsT=xT_sb[:, k,
                                                       tt * 128:(tt + 1) * 128],
                                            rhs=w1c[:, kk,
                                                    eb * 512:(eb + 1) * 512],
                                            start=(k == 0), stop=(k == KT - 1),
                                        )
                        # tt1 tail eb-major: each eb's pf1 slice
                        # finishes early enough for its bn_stats to overlap
                        # the remaining matmuls
                        for eb in range(4):
                            for ck in (7, 8, 9):
                                n = 4 if ck < 9 else 1
                                for kk in range(n):
                                    k = ck * 4 + kk
                                    nc.tensor.matmul(
                                        pf[1][:, eb, :],
                                        lhsT=xT_sb[:, k, 128:256],
                                        rhs=w1cs[ck][:, kk,
                                                     eb * 512:(eb + 1) * 512],
                                        start=(k == 0), stop=(k == KT - 1),
                                    )
                        mv0, rstd0, negmr0 = ln_stats(0, pf0)
                        # emb-stream gate: the et DMAs queue behind this
                        # gpsimd DMA, whose source is only written by the
                        # LN0 chain -- keeps the 3.8MB emb prefetch burst
                        # out of phase A's w1-stream window
                        nc.gpsimd.dma_start(embgate.ap()[0:1, :],
                                            negmr0[0:1, :])
                        ln_norm(0, 0, pf0, mv0, rstd0, negmr0)
                        ln_norm(0, 1, pf0, mv0, rstd0, negmr0)
                    mv1, rstd1, negmr1 = ln_stats(1, pf1)
                    ln_norm(1, 0, pf1, mv1, rstd1, negmr1)
                    ln_norm(1, 1, pf1, mv1, rstd1, negmr1)
                    with tc.tile_pool(name="psT0", bufs=2,
                                      space="PSUM") as psT0:
                        transp(0, 0, psT0)
                        transp(0, 1, psT0)
                        transp(1, 0, psT0)
                        transp(1, 1, psT0)

            # ------------- phase B: logits -> exp -> s, y ------------------
            with tc.tile_pool(name="psL", bufs=2, space="PSUM") as psL, \
                 tc.tile_pool(name="psY", bufs=4, space="PSUM") as psY, \
                 tc.tile_pool(name="psS", bufs=1, space="PSUM") as psS:
                r_acc = rp.tile([128, L], f32)
                y_acc = yap.tile([128, 2, 4, 512], f32)  # [t-part, tt, dq, d]
                rec_sb = rp.tile([128, 2], f32, tag="rec")
                pbar_t = [None, None]

                def logits_chunk(ci, v0, sz):
                    pbar = pbar_t[ci % 2]
                    for i in range(sz):
                        vt = v0 + i
                        w2t = w2p.tile([128, ET, 128], bf, tag="w2t",
                                       name=f"w2t{vt}")
                        nc.sync.dma_start(
                            w2t, w2s_ap[:, vt, :].rearrange(
                                "p (ko v) -> p ko v", v=128))
                        pl = psL.tile([128, L], f32, tag="pl",
                                      name=f"pl{vt}")
                        for ko in range(ET):
                            nc.tensor.matmul(
                                pl, lhsT=w2t[:, ko, :], rhs=fhatT[:, ko, :],
                                start=(ko == 0), stop=(ko == ET - 1))
                        nc.scalar.activation(out=pbar[:, i, :], in_=pl,
                                             func=AF.Exp, scale=1.0)
                        if vt == 0:
                            nc.vector.tensor_copy(out=r_acc,
                                                  in_=pbar[:, 0, :])
                        else:
                            nc.vector.tensor_tensor(
                                r_acc, pbar[:, i, :], r_acc, ALU.add)

                def s_finalize():
                    # finish softmax denominator: s[t] = ones^T @ r_acc
                    r_bf = rp.tile([128, L], bf, tag="r_bf")
                    nc.vector.tensor_copy(out=r_bf, in_=r_acc)
                    s_ps = psS.tile([128, 2], f32, tag="s_ps")
                    for tt in range(2):
                        nc.tensor.matmul(
                            s_ps[:, tt:tt + 1],
                            lhsT=r_bf[:, tt * 128:(tt + 1) * 128],
                            rhs=ones_t, start=True, stop=True)
                    nc.vector.reciprocal(out=rec_sb, in_=s_ps)

                def pe_group(ci, v0, sz, dq, first, last):
                    pbar = pbar_t[ci % 2]
                    ets = []
                    for g in range((sz + 4) // 5):
                        w = min(5, sz - 5 * g)
                        et = epool.tile([128, 5, 512], bf, tag="et",
                                        name=f"et{ci}{dq}{g}")
                        nc.gpsimd.dma_start(
                            et[:, 0:w, :],
                            emb_ap[:, dq, v0 + 5 * g:v0 + 5 * g + w, :])
                        ets.append(et)
                    py = [psY.tile([128, 512], f32, tag="py",
                                   name=f"py{ci}{dq}{tt}")
                          for tt in range(2)]
                    for i in range(sz):
                        for tt in range(2):
                            nc.tensor.matmul(
                                py[tt],
                                lhsT=pbar[:, i, tt * 128:(tt + 1) * 128],
                                rhs=ets[i // 5][:, i % 5, :],
                                start=(i == 0), stop=(i == sz - 1))
                    for tt in range(2):
                        if first:
                            nc.vector.tensor_copy(
                                out=y_acc[:, tt, dq, :], in_=py[tt])
                        elif not last:
                            nc.vector.tensor_tensor(
                                y_acc[:, tt, dq, :], py[tt],
                                y_acc[:, tt, dq, :], ALU.add)
                        else:
                            # epilogue: out = (y_acc + py) / s
                            yf = finp.tile([128, 512], f32, tag="yf",
                                           name=f"yf{dq}{tt}")
                            nc.vector.tensor_tensor(
                                yf, py[tt], y_acc[:, tt, dq, :], ALU.add)
                            osb = finp.tile([128, 512], f32, tag="osb",
                                            name=f"osb{dq}{tt}")
                            nc.vector.tensor_scalar_mul(
                                out=osb, in0=yf,
                                scalar1=rec_sb[:, tt:tt + 1])
                            # sync queue is idle by now (w2 done) and HWDGE
                            # has less fixed latency than the gpsimd SWDGE
                            nc.sync.dma_start(
                                out.ap()[tt * 128:(tt + 1) * 128,
                                         dq * 512:(dq + 1) * 512],
                                osb)

                # smaller first chunk softens the w2+emb DMA ramp right
                # after the transposes; smaller last chunk shortens the
                # exposed epilogue chain
                chunks = [(0, 15)] + [(15 + 25 * i, 25) for i in range(9)] \
                    + [(240, 10)]
                for ci, (v0, sz) in enumerate(chunks):
                    last = ci == len(chunks) - 1
                    pbar_t[ci % 2] = pbp.tile([128, CH, L], bf, tag="pbar",
                                              name=f"pbar{ci}")
                    logits_chunk(ci, v0, sz)
                    if last:
                        s_finalize()
                    for dq in range(4):
                        pe_group(ci, v0, sz, dq, ci == 0, last)

    nc.finalize()
    return nc


def _pixel_shuffle_np(x, s=S):
    b, seq, d = x.shape
    h = w = int(seq ** 0.5)
    x = x.reshape(b, h, w, d)
    x = x.reshape(b, h, w // s, d * s)
    x = x.transpose(0, 2, 1, 3)
    x = x.reshape(b, w // s, h // s, d * s * s)
    x = x.transpose(0, 2, 1, 3)
    return x.reshape(b, seq // (s * s), d * s * s)


def kernel(vision_feats, llm_token_embed, W1_w, W1_b, W2_w):
    global _BUILT, LAST_EXEC_TIME_NS
    _install_ntff_hook_shim()
    from concourse import bass_utils

    bf16 = ml_dtypes.bfloat16

    if _BUILT is None:
        _BUILT = _build()
    nc = _BUILT

    x = _pixel_shuffle_np(np.asarray(vision_feats, np.float32))  # (8,256,4608)

    w1T_h = np.zeros((KA, DT), bf16)
    w1T_h[:D4] = np.asarray(W1_w, np.float32).T.astype(bf16)
    w1T_h[D4] = np.asarray(W1_b, np.float32).astype(bf16)
    # repack to [p, ko, e] partition-contiguous
    w1s_h = np.ascontiguousarray(
        w1T_h.reshape(KT, 128, DT).transpose(1, 0, 2)).reshape(128, KT * DT)

    # w2s[p, vt, ko, v] = W2[vt*128+v, ko*128+p]
    w2s_h = np.ascontiguousarray(
        np.asarray(W2_w, np.float32).astype(bf16)
        .reshape(VT, 128, ET, 128).transpose(3, 0, 2, 1)
    ).reshape(128, VT * ET * 128)

    # repack to [p, dq, vt, d] partition-contiguous
    emb_h = np.ascontiguousarray(
        np.asarray(llm_token_embed, np.float32).astype(bf16)
        .reshape(VT, 128, 4, 512).transpose(1, 2, 0, 3)
    ).reshape(128, 4 * VT * 512)

    in_maps = []
    for c in range(N_CORES):
        xT_h = np.zeros((KA, L), bf16)
        xT_h[:D4] = x[c].T.astype(bf16)
        xT_h[D4] = 1.0
        xTs_h = np.ascontiguousarray(
            xT_h.reshape(KT, 128, L).transpose(1, 0, 2)).reshape(128, KT * L)
        in_maps.append({"xT": xTs_h, "w1T": w1s_h, "w2s": w2s_h,
                        "emb": emb_h})

    trace = bool(os.environ.get("KERNEL_TRACE"))
    kwargs = {}
    if trace:
        import tempfile

        kwargs["trace"] = True
        base = os.environ.get("KERNEL_TRACE_DIR")
        if base:
            os.makedirs(base, exist_ok=True)
            kwargs["tmpdir"] = tempfile.mkdtemp(dir=base)
        print("trace dir:", kwargs.get("tmpdir"), file=sys.stderr)

    res = bass_utils.run_bass_kernel_spmd(
        nc, in_maps, core_ids=list(range(N_CORES)), **kwargs)
    LAST_EXEC_TIME_NS = res.exec_time_ns

    out_full = np.stack(
        [np.asarray(res.results[c]["out"]) for c in range(N_CORES)], axis=0)
    return out_full.astype(np.float32)
